# revision 40
# baseline (speedup 1.0000x reference)
"""AttentionX Trainium2 kernel: 8-way head-parallel attention, v4.

Reference computation (B=1, N=2048, C_Q=256, H=8, C_HID=32):
    q = (q_x @ Wq) * 1/sqrt(32); k = kv_x @ Wk; v = kv_x @ Wv
    scores = q k^T + attn_bias; a = softmax(scores); o = a v
    out = (o * sigmoid(q_x @ Wg)) @ Wo

Sharding: one head per NeuronCore. Host combines: out = sum_h partial_h / sums_h.

Techniques:
  - Scores q k^T (K=32 contraction) via 2x row-tiled matmuls per
    half-group: tile_position (32i, 0) runs 2 k-blocks concurrently on
    32-row strips of the PE array (weights host-replicated across the
    partition groups, which is free since matmul cost is free-dim bound).
  - All bias data (8.4MB f16 per core) is preloaded into SBUF by 8 early
    1MB DMAs, so the main loop never waits on HBM.
  - The bias add is replaced by P = exp(scores) * exp(bias): the host
    precomputes exp(bias^T) slabs; on-chip application is a DVE f16
    tensor_mul at 2x rate (vs f32 PSUM adds at 1x). Score PSUM is
    double-buffered [128,1024] so ACT exp of one half-group overlaps the
    next half-group's score matmuls.
  - exp emitted with bias=-ln(16) for f16 range; cancels in the softmax
    quotient on the host.
  - sigmoid(x) = 0.5*(1+tanh(x/2)): tanh shares exp's ACT table set (one
    table load); 0.5 folded into Wo, +1 into a tensor_scalar.
  - PV is 2x column-tiled: even k-blocks accumulate into o0 (PSUM
    partitions 0:33), odd k-blocks into o1 (partitions 64:97), each with a
    ones column producing softmax denominators in rows 32/96.
  - Stage 3 (emitted inside the NEXT chunk to keep the PE queue fed) gates
    both o halves into og rows 0:33 / 64:97 with rows 33:64 zeroed, then a
    single K=97 matmul sums both halves through the augmented Wo whose
    column 256 passes the denominators -> one contiguous [128, 16*257]
    f16 output DMA per q-chunk, reshaped and divided on the host.
"""

import numpy as np

_STATE = {}

B, N, CQ, H, CH = 1, 2048, 256, 8, 32
NKB = N // 128  # 16 k-blocks of 128 keys
NQC = 4  # q-chunks of 512 queries
QC = N // NQC  # 512
HG = 8  # half-groups of 2 k-blocks per q-chunk
HW2 = N // 2  # 1024 score columns per half-group
LN16 = float(np.log(16.0))


def _build_nc():
    import concourse.bacc as bacc
    import concourse.tile as tile
    from concourse import mybir

    F32 = mybir.dt.float32
    F16 = mybir.dt.float16
    AF = mybir.ActivationFunctionType

    nc = bacc.Bacc("TRN2", target_bir_lowering=False, debug=False, num_devices=H)

    xq_d = nc.dram_tensor("xq", [128, 2 * N], F16, kind="ExternalInput")
    xkv_d = nc.dram_tensor("xkv", [128, 2 * N], F16, kind="ExternalInput")
    wq_d = nc.dram_tensor("wq", [128, 256], F16, kind="ExternalInput")
    wk_d = nc.dram_tensor("wk", [128, 256], F16, kind="ExternalInput")
    wg_d = nc.dram_tensor("wg", [128, 256], F16, kind="ExternalInput")
    wv_d = nc.dram_tensor("wv", [128, 64], F16, kind="ExternalInput")
    wo_d = nc.dram_tensor("wo", [128, 257], F16, kind="ExternalInput")
    eye_d = nc.dram_tensor("eye", [128, 128], F16, kind="ExternalInput")
    # 32 slabs of [128, 1024]: slab s=8c+hg; even hg raw bias, odd hg exp(bias)
    eb_d = nc.dram_tensor("eb", [128, 32 * HW2], F16, kind="ExternalInput")
    out_d = nc.dram_tensor("out", [128, 16 * 257], F16, kind="ExternalOutput")

    with tile.TileContext(nc) as tc:
        with (
            tc.tile_pool(name="const", bufs=1) as cpool,
            tc.tile_pool(name="proj", bufs=1) as ppool,
            tc.tile_pool(name="pexp", bufs=3) as pxpool,
            tc.tile_pool(name="pmul", bufs=3) as pmpool,
            tc.tile_pool(name="ogp", bufs=2) as ogpool,
            tc.tile_pool(name="outs", bufs=1) as opool,
        ):
            xq = cpool.tile([128, 2 * N], F16)
            nc.sync.dma_start(out=xq, in_=xq_d[:, :])
            xkv = cpool.tile([128, 2 * N], F16)
            nc.sync.dma_start(out=xkv, in_=xkv_d[:, :])
            wq = cpool.tile([128, 256], F16)
            nc.sync.dma_start(out=wq, in_=wq_d[:, :])
            wk = cpool.tile([128, 256], F16)
            nc.sync.dma_start(out=wk, in_=wk_d[:, :])
            wg = cpool.tile([128, 256], F16)
            nc.sync.dma_start(out=wg, in_=wg_d[:, :])
            wv = cpool.tile([128, 64], F16)
            nc.sync.dma_start(out=wv, in_=wv_d[:, :])
            wo = cpool.tile([128, 257], F16)
            nc.sync.dma_start(out=wo, in_=wo_d[:, :])
            eye = cpool.tile([128, 128], F16)
            nc.sync.dma_start(out=eye, in_=eye_d[:, :])
            # bias preload: one big SBUF buffer, 8 chunk DMAs (1MB each)
            ebsb = cpool.tile([128, 32 * HW2], F16)
            for cc in range(8):
                nc.sync.dma_start(
                    out=ebsb[:, 4 * HW2 * cc : 4 * HW2 * (cc + 1)],
                    in_=eb_d[:, 4 * HW2 * cc : 4 * HW2 * (cc + 1)],
                )

            nln16 = cpool.tile([128, 1], F32)
            nc.vector.memset(nln16, -LN16)

            qT4 = ppool.tile([128, N], F16, tag="qT4")
            kT4 = ppool.tile([128, N], F16, tag="kT4")
            gt4 = ppool.tile([128, N], F16, tag="gt4")
            tp1 = ppool.tile([128, N], F16, tag="tp1")
            vhat = ppool.tile([128, NKB * 33], F16, tag="vhat")
            outsb = opool.tile([128, 16 * 257], F16)

            nc.vector.memset(vhat, 1.0)
            nc.vector.memset(tp1[32:33, :], 1.0)
            nc.vector.memset(tp1[96:97, :], 1.0)

            # ---- stage 1: projections (full 128x128 PE mode) ----
            with (
                tc.tile_pool(name="proj_ps", bufs=2, space="PSUM") as proj_ps,
                tc.tile_pool(name="v_ps", bufs=2, space="PSUM") as v_ps,
                nc.named_scope("stage1_proj"),
            ):
                for w, src, dst in ((wq, xq, qT4), (wk, xkv, kT4)):
                    for f in range(4):
                        pp = proj_ps.tile([128, QC], F32, tag="pp")
                        nc.tensor.matmul(
                            pp, w[:, 0:128], src[:, QC * f : QC * (f + 1)],
                            start=True, stop=False,
                        )
                        nc.tensor.matmul(
                            pp, w[:, 128:256], src[:, N + QC * f : N + QC * (f + 1)],
                            start=False, stop=True,
                        )
                        nc.vector.tensor_copy(dst[:, QC * f : QC * (f + 1)], pp)
                # v projection, natural layout [seq, ch] + ones column
                for r in range(NKB):
                    vt = v_ps.tile([128, 32], F32, tag="v")
                    nc.tensor.matmul(
                        vt, xkv[:, 128 * r : 128 * (r + 1)], wv[:, 0:32],
                        start=True, stop=False,
                    )
                    nc.tensor.matmul(
                        vt, xkv[:, N + 128 * r : N + 128 * (r + 1)], wv[:, 32:64],
                        start=False, stop=True,
                    )
                    nc.vector.tensor_copy(vhat[:, 33 * r : 33 * r + 32], vt)
                # g projection -> tanh(0.5 x) (same ACT table set as exp)
                for f in range(4):
                    pp = proj_ps.tile([128, QC], F32, tag="pp")
                    nc.tensor.matmul(
                        pp, wg[:, 0:128], xq[:, QC * f : QC * (f + 1)],
                        start=True, stop=False,
                    )
                    nc.tensor.matmul(
                        pp, wg[:, 128:256], xq[:, N + QC * f : N + QC * (f + 1)],
                        start=False, stop=True,
                    )
                    nc.scalar.activation(
                        gt4[:, QC * f : QC * (f + 1)], pp, func=AF.Tanh, scale=0.5
                    )
                nc.vector.tensor_scalar_add(tp1[0:32, :], gt4[0:32, :], 1.0)
                nc.vector.tensor_scalar_add(tp1[64:96, :], gt4[64:96, :], 1.0)

            # ---- stage 2+3: attention main loop ----
            with (
                tc.tile_pool(name="sc_ps", bufs=2, space="PSUM") as sc_pool,
                tc.tile_pool(name="o_ps", bufs=1, space="PSUM") as o_pool,
                tc.tile_pool(name="s3_ps", bufs=2, space="PSUM") as s3_pool,
                nc.named_scope("stage2_attn"),
            ):
                o_tiles = {}

                def stage3(c):
                    # gating + output projection for finished q-chunk c;
                    # emitted inside the next chunk to keep the PE queue fed.
                    # og rows 0:33 = gated o0, rows 64:97 = gated o1, rows
                    # 33:64 zero -> one K=97 matmul sums both halves.
                    o0, o1 = o_tiles.pop(c)
                    og = ogpool.tile([128, QC], F16, tag="og")
                    nc.vector.memset(og[32:64, :], 0.0)
                    nc.vector.tensor_mul(
                        og[0:33, :], o0, tp1[0:33, QC * c : QC * (c + 1)]
                    )
                    nc.vector.tensor_mul(
                        og[64:97, :], o1, tp1[64:97, QC * c : QC * (c + 1)]
                    )
                    for j in range(4):
                        qb = 4 * c + j
                        s3 = s3_pool.tile([128, 257], F32, tag="s3")
                        nc.tensor.matmul(
                            s3, og[0:97, 128 * j : 128 * (j + 1)], wo[0:97, :],
                            start=True, stop=True,
                        )
                        nc.vector.tensor_copy(
                            outsb[:, 257 * qb : 257 * (qb + 1)], s3
                        )
                    nc.sync.dma_start(
                        out=out_d[:, 257 * 4 * c : 257 * 4 * (c + 1)],
                        in_=outsb[:, 257 * 4 * c : 257 * 4 * (c + 1)],
                    )

                for c in range(NQC):
                    o0_full = o_pool.tile([128, QC], F32, tag="o0")
                    o0 = o0_full[0:33, :]
                    o1_full = o_pool.tile([128, QC], F32, tag="o1")
                    o1 = o1_full[64:97, :]
                    o_tiles[c] = (o0, o1)
                    for hg in range(HG):
                        s = HG * c + hg
                        ebs = ebsb[:, HW2 * s : HW2 * (s + 1)]
                        sc = sc_pool.tile([128, HW2], F32, tag="sc")
                        for i in range(2):
                            kb = 2 * hg + i
                            nc.tensor.matmul(
                                sc[:, 512 * i : 512 * (i + 1)],
                                kT4[32 * i : 32 * (i + 1), 128 * kb : 128 * (kb + 1)],
                                qT4[32 * i : 32 * (i + 1), QC * c : QC * (c + 1)],
                                start=True, stop=True,
                                tile_position=(32 * i, 0),
                            )
                        if hg == 0 and c > 0:
                            stage3(c - 1)
                        pexp = pxpool.tile([128, HW2], F16, tag="pexp")
                        nc.scalar.activation(pexp, sc, func=AF.Exp, bias=nln16)
                        pt = pmpool.tile([128, HW2], F16, tag="p")
                        nc.vector.tensor_mul(pt, pexp, ebs)
                        for i in range(2):
                            kb = 2 * hg + i
                            nc.tensor.matmul(
                                o0 if i == 0 else o1,
                                vhat[:, 33 * kb : 33 * kb + 33],
                                pt[:, 512 * i : 512 * (i + 1)],
                                start=(hg == 0),
                                stop=(hg == HG - 1),
                                tile_position=(0, 0) if i == 0 else (0, 64),
                            )
                        nc.tensor.matmul(
                            o0_full[64:96, :],
                            vhat[:, 0:32],
                            qT4[:, QC * c : QC * (c + 1)],
                            start=False, stop=False,
                            skip_group_check=True,
                            tile_position=(0, 64),
                        )
                stage3(NQC - 1)

    nc.compile()
    return nc


def _get_nc():
    if "nc" not in _STATE:
        _STATE["nc"] = _build_nc()
    return _STATE["nc"]


def _pack2(m, dtype):
    """[256, X] -> [128, 2X]: c-chunk 0 in cols [0:X], chunk 1 in [X:2X]."""
    return np.ascontiguousarray(
        np.concatenate([m[0:128], m[128:256]], axis=1).astype(dtype)
    )


def kernel(q_x, kv_x, attn_bias, Wq, Wk, Wv, Wg, Wo):
    from concourse.bass_utils import run_bass_kernel_spmd

    BF = np.float16
    nc = _get_nc()

    q_x = np.asarray(q_x, dtype=np.float32)
    kv_x = np.asarray(kv_x, dtype=np.float32)
    attn_bias = np.asarray(attn_bias, dtype=np.float32)
    Wq = np.asarray(Wq, dtype=np.float32)
    Wk = np.asarray(Wk, dtype=np.float32)
    Wv = np.asarray(Wv, dtype=np.float32)
    Wg = np.asarray(Wg, dtype=np.float32)
    Wo = np.asarray(Wo, dtype=np.float32)

    xq = _pack2(np.ascontiguousarray(q_x[0].T), BF)
    xkv = _pack2(np.ascontiguousarray(kv_x[0].T), BF)
    eye = np.eye(128, dtype=BF)
    scale = np.float32(1.0 / np.sqrt(CH))

    in_maps = []
    for h in range(H):
        sl = slice(CH * h, CH * (h + 1))
        # 32 slabs [128, 1024], slab s=8c+hg covers q-chunk c, k-blocks
        # 2hg..2hg+1: slab[p, 512i+j] = bT[128*(2hg+i)+p, 512c+j].
        # Even hg slabs carry raw bias (PE eye-add), odd hg exp(bias) (DVE).
        bT = attn_bias[0, h].T.astype(np.float32)  # [keys, queries]
        slabs = (
            bT.reshape(8, 2, 128, 4, 512)  # hg, i, p, c, j
            .transpose(3, 0, 2, 1, 4)  # c, hg, p, i, j
            .reshape(32, 128, HW2)
        ).copy()
        slabs = np.exp(slabs)
        eb = np.ascontiguousarray(
            slabs.astype(BF).transpose(1, 0, 2).reshape(128, 32 * HW2)
        )
        woaug = np.zeros((128, 257), dtype=BF)
        woaug[0:32, 0:256] = (0.5 * Wo[sl, :]).astype(BF)
        woaug[32, 256] = 1.0
        woaug[64:96, 0:256] = woaug[0:32, 0:256]
        woaug[96, 256] = 1.0
        in_maps.append(
            {
                "xq": xq,
                "xkv": xkv,
                "wq": _pack2(np.tile(Wq[:, sl] * scale, (1, 4)), BF),
                "wk": _pack2(np.tile(Wk[:, sl], (1, 4)), BF),
                "wg": _pack2(np.tile(Wg[:, sl], (1, 4)), BF),
                "wv": _pack2(Wv[:, sl], BF),
                "wo": woaug,
                "eye": eye,
                "eb": eb,
            }
        )

    res = run_bass_kernel_spmd(nc, in_maps, list(range(H)))

    out = np.zeros((N, CQ), dtype=np.float32)
    for h in range(H):
        full = (
            res.results[h]["out"]
            .astype(np.float32)
            .reshape(128, 16, 257)
            .transpose(1, 0, 2)
            .reshape(N, 257)
        )
        out += full[:, 0:256] / full[:, 256][:, None]
    return out.reshape(B, N, CQ).astype(np.float32)


# revision 41
# speedup vs baseline: 1.1877x; 1.1877x over previous
"""AttentionX Trainium2 kernel: 8-way head-parallel attention, v4.

Reference computation (B=1, N=2048, C_Q=256, H=8, C_HID=32):
    q = (q_x @ Wq) * 1/sqrt(32); k = kv_x @ Wk; v = kv_x @ Wv
    scores = q k^T + attn_bias; a = softmax(scores); o = a v
    out = (o * sigmoid(q_x @ Wg)) @ Wo

Sharding: one head per NeuronCore. Host combines: out = sum_h partial_h / sums_h.

Techniques:
  - Scores q k^T (K=32 contraction) via 2x row-tiled matmuls per
    half-group: tile_position (32i, 0) runs 2 k-blocks concurrently on
    32-row strips of the PE array (weights host-replicated across the
    partition groups, which is free since matmul cost is free-dim bound).
  - All bias data (8.4MB f16 per core) is preloaded into SBUF by 8 early
    1MB DMAs, so the main loop never waits on HBM.
  - The bias add is replaced by P = exp(scores) * exp(bias): the host
    precomputes exp(bias^T) slabs; on-chip application is a DVE f16
    tensor_mul at 2x rate (vs f32 PSUM adds at 1x). Score PSUM is
    double-buffered [128,1024] so ACT exp of one half-group overlaps the
    next half-group's score matmuls.
  - exp emitted with bias=-ln(16) for f16 range; cancels in the softmax
    quotient on the host.
  - sigmoid(x) = 0.5*(1+tanh(x/2)): tanh shares exp's ACT table set (one
    table load); 0.5 folded into Wo, +1 into a tensor_scalar.
  - PV is 2x column-tiled: even k-blocks accumulate into o0 (PSUM
    partitions 0:33), odd k-blocks into o1 (partitions 64:97), each with a
    ones column producing softmax denominators in rows 32/96.
  - Stage 3 (emitted inside the NEXT chunk to keep the PE queue fed) gates
    both o halves into og rows 0:33 / 64:97 with rows 33:64 zeroed, then a
    single K=97 matmul sums both halves through the augmented Wo whose
    column 256 passes the denominators -> one contiguous [128, 16*257]
    f16 output DMA per q-chunk, reshaped and divided on the host.
"""

import numpy as np

_STATE = {}

B, N, CQ, H, CH = 1, 2048, 256, 8, 32
NKB = N // 128  # 16 k-blocks of 128 keys
NQC = 4  # q-chunks of 512 queries
QC = N // NQC  # 512
HG = 8  # half-groups of 2 k-blocks per q-chunk
HW2 = N // 2  # 1024 score columns per half-group
LN16 = float(np.log(16.0))


def _build_nc():
    import concourse.bacc as bacc
    import concourse.tile as tile
    from concourse import mybir

    F32 = mybir.dt.float32
    F16 = mybir.dt.float16
    AF = mybir.ActivationFunctionType

    nc = bacc.Bacc("TRN2", target_bir_lowering=False, debug=False, num_devices=H)

    xq_d = nc.dram_tensor("xq", [128, 2 * N], F16, kind="ExternalInput")
    xkv_d = nc.dram_tensor("xkv", [128, 2 * N], F16, kind="ExternalInput")
    wq_d = nc.dram_tensor("wq", [128, 256], F16, kind="ExternalInput")
    wk_d = nc.dram_tensor("wk", [128, 256], F16, kind="ExternalInput")
    wg_d = nc.dram_tensor("wg", [128, 256], F16, kind="ExternalInput")
    wv_d = nc.dram_tensor("wv", [128, 64], F16, kind="ExternalInput")
    wo_d = nc.dram_tensor("wo", [128, 257], F16, kind="ExternalInput")
    eye_d = nc.dram_tensor("eye", [128, 128], F16, kind="ExternalInput")
    # 32 slabs of [128, 1024]: slab s=8c+hg; even hg raw bias, odd hg exp(bias)
    eb_d = nc.dram_tensor("eb", [128, 32 * HW2], F16, kind="ExternalInput")
    out_d = nc.dram_tensor("out", [128, 16 * 257], F16, kind="ExternalOutput")

    with tile.TileContext(nc) as tc:
        with (
            tc.tile_pool(name="const", bufs=1) as cpool,
            tc.tile_pool(name="proj", bufs=1) as ppool,
            tc.tile_pool(name="pexp", bufs=2) as pxpool,
            tc.tile_pool(name="pmul", bufs=3) as pmpool,
            tc.tile_pool(name="ogp", bufs=2) as ogpool,
            tc.tile_pool(name="outs", bufs=1) as opool,
        ):
            xq = cpool.tile([128, 2 * N], F16)
            nc.sync.dma_start(out=xq, in_=xq_d[:, :])
            xkv = cpool.tile([128, 2 * N], F16)
            nc.sync.dma_start(out=xkv, in_=xkv_d[:, :])
            wq = cpool.tile([128, 256], F16)
            nc.sync.dma_start(out=wq, in_=wq_d[:, :])
            wk = cpool.tile([128, 256], F16)
            nc.sync.dma_start(out=wk, in_=wk_d[:, :])
            wg = cpool.tile([128, 256], F16)
            nc.sync.dma_start(out=wg, in_=wg_d[:, :])
            wv = cpool.tile([128, 64], F16)
            nc.sync.dma_start(out=wv, in_=wv_d[:, :])
            wo = cpool.tile([128, 257], F16)
            nc.sync.dma_start(out=wo, in_=wo_d[:, :])
            eye = cpool.tile([128, 128], F16)
            nc.sync.dma_start(out=eye, in_=eye_d[:, :])
            # bias preload: one big SBUF buffer, 8 chunk DMAs (1MB each)
            ebsb = cpool.tile([128, 32 * HW2], F16)
            for cc in range(8):
                nc.sync.dma_start(
                    out=ebsb[:, 4 * HW2 * cc : 4 * HW2 * (cc + 1)],
                    in_=eb_d[:, 4 * HW2 * cc : 4 * HW2 * (cc + 1)],
                )

            nln16 = cpool.tile([128, 1], F32)
            nc.vector.memset(nln16, -LN16)

            qT4 = ppool.tile([128, N], F16, tag="qT4")
            kT4 = ppool.tile([128, N], F16, tag="kT4")
            gt4 = ppool.tile([128, N], F16, tag="gt4")
            tp1 = ppool.tile([128, N], F16, tag="tp1")
            vhat = ppool.tile([128, NKB * 33], F16, tag="vhat")
            outsb = opool.tile([128, 16 * 257], F16)

            nc.vector.memset(vhat, 1.0)
            nc.vector.memset(tp1[32:33, :], 1.0)
            nc.vector.memset(tp1[96:97, :], 1.0)

            # ---- stage 1: projections (full 128x128 PE mode) ----
            with (
                tc.tile_pool(name="proj_ps", bufs=2, space="PSUM") as proj_ps,
                tc.tile_pool(name="v_ps", bufs=2, space="PSUM") as v_ps,
                nc.named_scope("stage1_proj"),
            ):
                for w, src, dst in ((wq, xq, qT4), (wk, xkv, kT4)):
                    for f in range(4):
                        pp = proj_ps.tile([128, QC], F32, tag="pp")
                        nc.tensor.matmul(
                            pp, w[:, 0:128], src[:, QC * f : QC * (f + 1)],
                            start=True, stop=False,
                        )
                        nc.tensor.matmul(
                            pp, w[:, 128:256], src[:, N + QC * f : N + QC * (f + 1)],
                            start=False, stop=True,
                        )
                        nc.vector.tensor_copy(dst[:, QC * f : QC * (f + 1)], pp)
                # v projection, natural layout [seq, ch] + ones column
                for r in range(NKB):
                    vt = v_ps.tile([128, 32], F32, tag="v")
                    nc.tensor.matmul(
                        vt, xkv[:, 128 * r : 128 * (r + 1)], wv[:, 0:32],
                        start=True, stop=False,
                    )
                    nc.tensor.matmul(
                        vt, xkv[:, N + 128 * r : N + 128 * (r + 1)], wv[:, 32:64],
                        start=False, stop=True,
                    )
                    nc.vector.tensor_copy(vhat[:, 33 * r : 33 * r + 32], vt)
                # g projection -> tanh(0.5 x) (same ACT table set as exp)
                for f in range(4):
                    pp = proj_ps.tile([128, QC], F32, tag="pp")
                    nc.tensor.matmul(
                        pp, wg[:, 0:128], xq[:, QC * f : QC * (f + 1)],
                        start=True, stop=False,
                    )
                    nc.tensor.matmul(
                        pp, wg[:, 128:256], xq[:, N + QC * f : N + QC * (f + 1)],
                        start=False, stop=True,
                    )
                    nc.scalar.activation(
                        gt4[:, QC * f : QC * (f + 1)], pp, func=AF.Tanh, scale=0.5
                    )
                nc.vector.tensor_scalar_add(tp1[0:32, :], gt4[0:32, :], 1.0)
                nc.vector.tensor_scalar_add(tp1[64:96, :], gt4[64:96, :], 1.0)

            # ---- stage 2+3: attention main loop ----
            with (
                tc.tile_pool(name="sc_ps", bufs=2, space="PSUM") as sc_pool,
                tc.tile_pool(name="o_ps", bufs=1, space="PSUM") as o_pool,
                tc.tile_pool(name="s3_ps", bufs=2, space="PSUM") as s3_pool,
                nc.named_scope("stage2_attn"),
            ):
                o_tiles = {}

                def stage3(c):
                    # gating + output projection for finished q-chunk c;
                    # emitted inside the next chunk to keep the PE queue fed.
                    # og rows 0:33 = gated o0, rows 64:97 = gated o1, rows
                    # 33:64 zero -> one K=97 matmul sums both halves.
                    o0, o1 = o_tiles.pop(c)
                    og = ogpool.tile([128, QC], F16, tag="og")
                    nc.vector.memset(og[32:64, :], 0.0)
                    nc.vector.tensor_mul(
                        og[0:33, :], o0, tp1[0:33, QC * c : QC * (c + 1)]
                    )
                    nc.vector.tensor_mul(
                        og[64:97, :], o1, tp1[64:97, QC * c : QC * (c + 1)]
                    )
                    for j in range(4):
                        qb = 4 * c + j
                        s3 = s3_pool.tile([128, 257], F32, tag="s3")
                        nc.tensor.matmul(
                            s3, og[0:97, 128 * j : 128 * (j + 1)], wo[0:97, :],
                            start=True, stop=True,
                        )
                        nc.vector.tensor_copy(
                            outsb[:, 257 * qb : 257 * (qb + 1)], s3
                        )
                    nc.sync.dma_start(
                        out=out_d[:, 257 * 4 * c : 257 * 4 * (c + 1)],
                        in_=outsb[:, 257 * 4 * c : 257 * 4 * (c + 1)],
                    )

                for c in range(NQC):
                    o0 = o_pool.tile([33, QC], F32, tag="o0")
                    o1_full = o_pool.tile([128, QC], F32, tag="o1")
                    o1 = o1_full[64:97, :]
                    o_tiles[c] = (o0, o1)
                    for hg in range(HG):
                        s = HG * c + hg
                        ebs = ebsb[:, HW2 * s : HW2 * (s + 1)]
                        sc = sc_pool.tile([128, HW2], F32, tag="sc")
                        for i in range(2):
                            kb = 2 * hg + i
                            nc.tensor.matmul(
                                sc[:, 512 * i : 512 * (i + 1)],
                                kT4[32 * i : 32 * (i + 1), 128 * kb : 128 * (kb + 1)],
                                qT4[32 * i : 32 * (i + 1), QC * c : QC * (c + 1)],
                                start=True, stop=True,
                                tile_position=(32 * i, 0),
                            )
                        if hg == 0 and c > 0:
                            stage3(c - 1)
                        pexp = pxpool.tile([128, HW2], F16, tag="pexp")
                        nc.scalar.activation(pexp, sc, func=AF.Exp, bias=nln16)
                        pt = pmpool.tile([128, HW2], F16, tag="p")
                        nc.vector.tensor_mul(pt, pexp, ebs)
                        for i in range(2):
                            kb = 2 * hg + i
                            nc.tensor.matmul(
                                o0 if i == 0 else o1,
                                vhat[:, 33 * kb : 33 * kb + 33],
                                pt[:, 512 * i : 512 * (i + 1)],
                                start=(hg == 0),
                                stop=(hg == HG - 1),
                                tile_position=(0, 0) if i == 0 else (0, 64),
                            )
                stage3(NQC - 1)

    nc.compile()
    return nc


def _get_nc():
    if "nc" not in _STATE:
        _STATE["nc"] = _build_nc()
    return _STATE["nc"]


def _pack2(m, dtype):
    """[256, X] -> [128, 2X]: c-chunk 0 in cols [0:X], chunk 1 in [X:2X]."""
    return np.ascontiguousarray(
        np.concatenate([m[0:128], m[128:256]], axis=1).astype(dtype)
    )


def kernel(q_x, kv_x, attn_bias, Wq, Wk, Wv, Wg, Wo):
    from concourse.bass_utils import run_bass_kernel_spmd

    BF = np.float16
    nc = _get_nc()

    q_x = np.asarray(q_x, dtype=np.float32)
    kv_x = np.asarray(kv_x, dtype=np.float32)
    attn_bias = np.asarray(attn_bias, dtype=np.float32)
    Wq = np.asarray(Wq, dtype=np.float32)
    Wk = np.asarray(Wk, dtype=np.float32)
    Wv = np.asarray(Wv, dtype=np.float32)
    Wg = np.asarray(Wg, dtype=np.float32)
    Wo = np.asarray(Wo, dtype=np.float32)

    xq = _pack2(np.ascontiguousarray(q_x[0].T), BF)
    xkv = _pack2(np.ascontiguousarray(kv_x[0].T), BF)
    eye = np.eye(128, dtype=BF)
    scale = np.float32(1.0 / np.sqrt(CH))

    in_maps = []
    for h in range(H):
        sl = slice(CH * h, CH * (h + 1))
        # 32 slabs [128, 1024], slab s=8c+hg covers q-chunk c, k-blocks
        # 2hg..2hg+1: slab[p, 512i+j] = bT[128*(2hg+i)+p, 512c+j].
        # Even hg slabs carry raw bias (PE eye-add), odd hg exp(bias) (DVE).
        bT = attn_bias[0, h].T.astype(np.float32)  # [keys, queries]
        slabs = (
            bT.reshape(8, 2, 128, 4, 512)  # hg, i, p, c, j
            .transpose(3, 0, 2, 1, 4)  # c, hg, p, i, j
            .reshape(32, 128, HW2)
        ).copy()
        slabs = np.exp(slabs)
        eb = np.ascontiguousarray(
            slabs.astype(BF).transpose(1, 0, 2).reshape(128, 32 * HW2)
        )
        woaug = np.zeros((128, 257), dtype=BF)
        woaug[0:32, 0:256] = (0.5 * Wo[sl, :]).astype(BF)
        woaug[32, 256] = 1.0
        woaug[64:96, 0:256] = woaug[0:32, 0:256]
        woaug[96, 256] = 1.0
        in_maps.append(
            {
                "xq": xq,
                "xkv": xkv,
                "wq": _pack2(np.tile(Wq[:, sl] * scale, (1, 4)), BF),
                "wk": _pack2(np.tile(Wk[:, sl], (1, 4)), BF),
                "wg": _pack2(np.tile(Wg[:, sl], (1, 4)), BF),
                "wv": _pack2(Wv[:, sl], BF),
                "wo": woaug,
                "eye": eye,
                "eb": eb,
            }
        )

    res = run_bass_kernel_spmd(nc, in_maps, list(range(H)))

    out = np.zeros((N, CQ), dtype=np.float32)
    for h in range(H):
        full = (
            res.results[h]["out"]
            .astype(np.float32)
            .reshape(128, 16, 257)
            .transpose(1, 0, 2)
            .reshape(N, 257)
        )
        out += full[:, 0:256] / full[:, 256][:, None]
    return out.reshape(B, N, CQ).astype(np.float32)


# revision 42
# speedup vs baseline: 1.3794x; 1.1615x over previous
"""AttentionX Trainium2 kernel: 8-way head-parallel attention, v4.

Reference computation (B=1, N=2048, C_Q=256, H=8, C_HID=32):
    q = (q_x @ Wq) * 1/sqrt(32); k = kv_x @ Wk; v = kv_x @ Wv
    scores = q k^T + attn_bias; a = softmax(scores); o = a v
    out = (o * sigmoid(q_x @ Wg)) @ Wo

Sharding: one head per NeuronCore. Host combines: out = sum_h partial_h / sums_h.

Techniques:
  - Scores q k^T (K=32 contraction) via 2x row-tiled matmuls per
    half-group: tile_position (32i, 0) runs 2 k-blocks concurrently on
    32-row strips of the PE array (weights host-replicated across the
    partition groups, which is free since matmul cost is free-dim bound).
  - All bias data (8.4MB f16 per core) is preloaded into SBUF by 8 early
    1MB DMAs, so the main loop never waits on HBM.
  - The bias add is replaced by P = exp(scores) * exp(bias): the host
    precomputes exp(bias^T) slabs; on-chip application is a DVE f16
    tensor_mul at 2x rate (vs f32 PSUM adds at 1x). Score PSUM is
    double-buffered [128,1024] so ACT exp of one half-group overlaps the
    next half-group's score matmuls.
  - exp emitted with bias=-ln(16) for f16 range; cancels in the softmax
    quotient on the host.
  - sigmoid(x) = 0.5*(1+tanh(x/2)): tanh shares exp's ACT table set (one
    table load); 0.5 folded into Wo, +1 into a tensor_scalar.
  - PV is 2x column-tiled: even k-blocks accumulate into o0 (PSUM
    partitions 0:33), odd k-blocks into o1 (partitions 64:97), each with a
    ones column producing softmax denominators in rows 32/96.
  - Stage 3 (emitted inside the NEXT chunk to keep the PE queue fed) gates
    both o halves into og rows 0:33 / 64:97 with rows 33:64 zeroed, then a
    single K=97 matmul sums both halves through the augmented Wo whose
    column 256 passes the denominators -> one contiguous [128, 16*257]
    f16 output DMA per q-chunk, reshaped and divided on the host.
"""

import numpy as np

_STATE = {}

B, N, CQ, H, CH = 1, 2048, 256, 8, 32
NKB = N // 128  # 16 k-blocks of 128 keys
NQC = 4  # q-chunks of 512 queries
QC = N // NQC  # 512
HG = 8  # half-groups of 2 k-blocks per q-chunk
HW2 = N // 2  # 1024 score columns per half-group
LN16 = float(np.log(16.0))


def _build_nc():
    import concourse.bacc as bacc
    import concourse.tile as tile
    from concourse import mybir

    F32 = mybir.dt.float32
    F16 = mybir.dt.float16
    AF = mybir.ActivationFunctionType

    nc = bacc.Bacc("TRN2", target_bir_lowering=False, debug=False, num_devices=H)

    xq_d = nc.dram_tensor("xq", [128, 2 * N], F16, kind="ExternalInput")
    xkv_d = nc.dram_tensor("xkv", [128, 2 * N], F16, kind="ExternalInput")
    wq_d = nc.dram_tensor("wq", [128, 256], F16, kind="ExternalInput")
    wk_d = nc.dram_tensor("wk", [128, 256], F16, kind="ExternalInput")
    wg_d = nc.dram_tensor("wg", [128, 256], F16, kind="ExternalInput")
    wv_d = nc.dram_tensor("wv", [128, 64], F16, kind="ExternalInput")
    wo_d = nc.dram_tensor("wo", [128, 257], F16, kind="ExternalInput")
    # 32 slabs of [128, 1024]: slab s=8c+hg; even hg raw bias, odd hg exp(bias)
    eb_d = nc.dram_tensor("eb", [128, 32 * HW2], F16, kind="ExternalInput")
    out_d = nc.dram_tensor("out", [128, 16 * 257], F16, kind="ExternalOutput")

    with tile.TileContext(nc) as tc:
        with (
            tc.tile_pool(name="const", bufs=1) as cpool,
            tc.tile_pool(name="proj", bufs=1) as ppool,
            tc.tile_pool(name="pexp", bufs=2) as pxpool,
            tc.tile_pool(name="pmul", bufs=3) as pmpool,
            tc.tile_pool(name="ogp", bufs=2) as ogpool,
            tc.tile_pool(name="outs", bufs=1) as opool,
        ):
            xq = cpool.tile([128, 2 * N], F16)
            nc.sync.dma_start(out=xq, in_=xq_d[:, :])
            xkv = cpool.tile([128, 2 * N], F16)
            nc.sync.dma_start(out=xkv, in_=xkv_d[:, :])
            wq = cpool.tile([128, 256], F16)
            nc.sync.dma_start(out=wq, in_=wq_d[:, :])
            wk = cpool.tile([128, 256], F16)
            nc.sync.dma_start(out=wk, in_=wk_d[:, :])
            wg = cpool.tile([128, 256], F16)
            nc.sync.dma_start(out=wg, in_=wg_d[:, :])
            wv = cpool.tile([128, 64], F16)
            nc.sync.dma_start(out=wv, in_=wv_d[:, :])
            wo = cpool.tile([128, 257], F16)
            nc.sync.dma_start(out=wo, in_=wo_d[:, :])
            # bias preload: one big SBUF buffer, 8 chunk DMAs (1MB each)
            ebsb = cpool.tile([128, 32 * HW2], F16)
            for cc in range(8):
                nc.sync.dma_start(
                    out=ebsb[:, 4 * HW2 * cc : 4 * HW2 * (cc + 1)],
                    in_=eb_d[:, 4 * HW2 * cc : 4 * HW2 * (cc + 1)],
                )

            nln16 = cpool.tile([128, 1], F32)
            nc.vector.memset(nln16, -LN16)
            actwarm = cpool.tile([128, 1], F32)
            nc.scalar.activation(actwarm, nln16, func=AF.Exp)

            qT4 = ppool.tile([128, N], F16, tag="qT4")
            kT4 = ppool.tile([128, N], F16, tag="kT4")
            gt4 = ppool.tile([128, N], F16, tag="gt4")
            tp1 = ppool.tile([128, N], F16, tag="tp1")
            vhat = ppool.tile([128, NKB * 33], F16, tag="vhat")
            outsb = opool.tile([128, 16 * 257], F16)

            nc.vector.memset(vhat, 1.0)
            nc.vector.memset(tp1[32:33, :], 1.0)
            nc.vector.memset(tp1[96:97, :], 1.0)

            # ---- stage 1: projections (full 128x128 PE mode) ----
            with (
                tc.tile_pool(name="proj_ps", bufs=2, space="PSUM") as proj_ps,
                tc.tile_pool(name="v_ps", bufs=2, space="PSUM") as v_ps,
                nc.named_scope("stage1_proj"),
            ):
                for w, src, dst in ((wq, xq, qT4), (wk, xkv, kT4)):
                    for f in range(4):
                        pp = proj_ps.tile([128, QC], F32, tag="pp")
                        nc.tensor.matmul(
                            pp, w[:, 0:128], src[:, QC * f : QC * (f + 1)],
                            start=True, stop=False,
                        )
                        nc.tensor.matmul(
                            pp, w[:, 128:256], src[:, N + QC * f : N + QC * (f + 1)],
                            start=False, stop=True,
                        )
                        nc.vector.tensor_copy(dst[:, QC * f : QC * (f + 1)], pp)
                # v projection, natural layout [seq, ch] + ones column
                for r in range(NKB):
                    vt = v_ps.tile([128, 32], F32, tag="v")
                    nc.tensor.matmul(
                        vt, xkv[:, 128 * r : 128 * (r + 1)], wv[:, 0:32],
                        start=True, stop=False,
                    )
                    nc.tensor.matmul(
                        vt, xkv[:, N + 128 * r : N + 128 * (r + 1)], wv[:, 32:64],
                        start=False, stop=True,
                    )
                    nc.vector.tensor_copy(vhat[:, 33 * r : 33 * r + 32], vt)
                # g projection -> tanh(0.5 x) (same ACT table set as exp)
                for f in range(4):
                    pp = proj_ps.tile([128, QC], F32, tag="pp")
                    nc.tensor.matmul(
                        pp, wg[:, 0:128], xq[:, QC * f : QC * (f + 1)],
                        start=True, stop=False,
                    )
                    nc.tensor.matmul(
                        pp, wg[:, 128:256], xq[:, N + QC * f : N + QC * (f + 1)],
                        start=False, stop=True,
                    )
                    nc.scalar.activation(
                        gt4[:, QC * f : QC * (f + 1)], pp, func=AF.Tanh, scale=0.5
                    )
                nc.vector.tensor_scalar_add(tp1[0:32, :], gt4[0:32, :], 1.0)
                nc.vector.tensor_scalar_add(tp1[64:96, :], gt4[64:96, :], 1.0)

            # ---- stage 2+3: attention main loop ----
            with (
                tc.tile_pool(name="sc_ps", bufs=2, space="PSUM") as sc_pool,
                tc.tile_pool(name="o_ps", bufs=1, space="PSUM") as o_pool,
                tc.tile_pool(name="s3_ps", bufs=2, space="PSUM") as s3_pool,
                nc.named_scope("stage2_attn"),
            ):
                o_tiles = {}

                def stage3(c):
                    # gating + output projection for finished q-chunk c;
                    # emitted inside the next chunk to keep the PE queue fed.
                    # og rows 0:33 = gated o0, rows 64:97 = gated o1, rows
                    # 33:64 zero -> one K=97 matmul sums both halves.
                    o0, o1 = o_tiles.pop(c)
                    og = ogpool.tile([128, QC], F16, tag="og")
                    nc.vector.memset(og[32:64, :], 0.0)
                    nc.vector.tensor_mul(
                        og[0:33, :], o0, tp1[0:33, QC * c : QC * (c + 1)]
                    )
                    nc.vector.tensor_mul(
                        og[64:97, :], o1, tp1[64:97, QC * c : QC * (c + 1)]
                    )
                    for j in range(4):
                        qb = 4 * c + j
                        s3 = s3_pool.tile([128, 257], F32, tag="s3")
                        nc.tensor.matmul(
                            s3, og[0:97, 128 * j : 128 * (j + 1)], wo[0:97, :],
                            start=True, stop=True,
                        )
                        nc.vector.tensor_copy(
                            outsb[:, 257 * qb : 257 * (qb + 1)], s3
                        )
                    nc.sync.dma_start(
                        out=out_d[:, 257 * 4 * c : 257 * 4 * (c + 1)],
                        in_=outsb[:, 257 * 4 * c : 257 * 4 * (c + 1)],
                    )

                for c in range(NQC):
                    o0 = o_pool.tile([33, QC], F32, tag="o0")
                    o1_full = o_pool.tile([128, QC], F32, tag="o1")
                    o1 = o1_full[64:97, :]
                    o_tiles[c] = (o0, o1)
                    for hg in range(HG):
                        s = HG * c + hg
                        ebs = ebsb[:, HW2 * s : HW2 * (s + 1)]
                        sc = sc_pool.tile([128, HW2], F32, tag="sc")
                        for i in range(2):
                            kb = 2 * hg + i
                            nc.tensor.matmul(
                                sc[:, 512 * i : 512 * (i + 1)],
                                kT4[32 * i : 32 * (i + 1), 128 * kb : 128 * (kb + 1)],
                                qT4[32 * i : 32 * (i + 1), QC * c : QC * (c + 1)],
                                start=True, stop=True,
                                tile_position=(32 * i, 0),
                            )
                        if hg == 0 and c > 0:
                            stage3(c - 1)
                        pexp = pxpool.tile([128, HW2], F16, tag="pexp")
                        nc.scalar.activation(pexp, sc, func=AF.Exp, bias=nln16)
                        pt = pmpool.tile([128, HW2], F16, tag="p")
                        nc.vector.tensor_mul(pt, pexp, ebs)
                        for i in range(2):
                            kb = 2 * hg + i
                            nc.tensor.matmul(
                                o0 if i == 0 else o1,
                                vhat[:, 33 * kb : 33 * kb + 33],
                                pt[:, 512 * i : 512 * (i + 1)],
                                start=(hg == 0),
                                stop=(hg == HG - 1),
                                tile_position=(0, 0) if i == 0 else (0, 64),
                            )
                stage3(NQC - 1)

    nc.compile()
    return nc


def _get_nc():
    if "nc" not in _STATE:
        _STATE["nc"] = _build_nc()
    return _STATE["nc"]


def _pack2(m, dtype):
    """[256, X] -> [128, 2X]: c-chunk 0 in cols [0:X], chunk 1 in [X:2X]."""
    return np.ascontiguousarray(
        np.concatenate([m[0:128], m[128:256]], axis=1).astype(dtype)
    )


def kernel(q_x, kv_x, attn_bias, Wq, Wk, Wv, Wg, Wo):
    from concourse.bass_utils import run_bass_kernel_spmd

    BF = np.float16
    nc = _get_nc()

    q_x = np.asarray(q_x, dtype=np.float32)
    kv_x = np.asarray(kv_x, dtype=np.float32)
    attn_bias = np.asarray(attn_bias, dtype=np.float32)
    Wq = np.asarray(Wq, dtype=np.float32)
    Wk = np.asarray(Wk, dtype=np.float32)
    Wv = np.asarray(Wv, dtype=np.float32)
    Wg = np.asarray(Wg, dtype=np.float32)
    Wo = np.asarray(Wo, dtype=np.float32)

    xq = _pack2(np.ascontiguousarray(q_x[0].T), BF)
    xkv = _pack2(np.ascontiguousarray(kv_x[0].T), BF)
    scale = np.float32(1.0 / np.sqrt(CH))

    in_maps = []
    for h in range(H):
        sl = slice(CH * h, CH * (h + 1))
        # 32 slabs [128, 1024], slab s=8c+hg covers q-chunk c, k-blocks
        # 2hg..2hg+1: slab[p, 512i+j] = bT[128*(2hg+i)+p, 512c+j].
        # Even hg slabs carry raw bias (PE eye-add), odd hg exp(bias) (DVE).
        bT = attn_bias[0, h].T.astype(np.float32)  # [keys, queries]
        slabs = (
            bT.reshape(8, 2, 128, 4, 512)  # hg, i, p, c, j
            .transpose(3, 0, 2, 1, 4)  # c, hg, p, i, j
            .reshape(32, 128, HW2)
        ).copy()
        slabs = np.exp(slabs)
        eb = np.ascontiguousarray(
            slabs.astype(BF).transpose(1, 0, 2).reshape(128, 32 * HW2)
        )
        woaug = np.zeros((128, 257), dtype=BF)
        woaug[0:32, 0:256] = (0.5 * Wo[sl, :]).astype(BF)
        woaug[32, 256] = 1.0
        woaug[64:96, 0:256] = woaug[0:32, 0:256]
        woaug[96, 256] = 1.0
        in_maps.append(
            {
                "xq": xq,
                "xkv": xkv,
                "wq": _pack2(np.tile(Wq[:, sl] * scale, (1, 4)), BF),
                "wk": _pack2(np.tile(Wk[:, sl], (1, 4)), BF),
                "wg": _pack2(np.tile(Wg[:, sl], (1, 4)), BF),
                "wv": _pack2(Wv[:, sl], BF),
                "wo": woaug,
                "eb": eb,
            }
        )

    res = run_bass_kernel_spmd(nc, in_maps, list(range(H)))

    out = np.zeros((N, CQ), dtype=np.float32)
    for h in range(H):
        full = (
            res.results[h]["out"]
            .astype(np.float32)
            .reshape(128, 16, 257)
            .transpose(1, 0, 2)
            .reshape(N, 257)
        )
        out += full[:, 0:256] / full[:, 256][:, None]
    return out.reshape(B, N, CQ).astype(np.float32)


# revision 44
# speedup vs baseline: 1.3991x; 1.0142x over previous
"""AttentionX Trainium2 kernel: 8-way head-parallel attention, v4.

Reference computation (B=1, N=2048, C_Q=256, H=8, C_HID=32):
    q = (q_x @ Wq) * 1/sqrt(32); k = kv_x @ Wk; v = kv_x @ Wv
    scores = q k^T + attn_bias; a = softmax(scores); o = a v
    out = (o * sigmoid(q_x @ Wg)) @ Wo

Sharding: one head per NeuronCore. Host combines: out = sum_h partial_h / sums_h.

Techniques:
  - Scores q k^T (K=32 contraction) via 2x row-tiled matmuls per
    half-group: tile_position (32i, 0) runs 2 k-blocks concurrently on
    32-row strips of the PE array (weights host-replicated across the
    partition groups, which is free since matmul cost is free-dim bound).
  - All bias data (8.4MB f16 per core) is preloaded into SBUF by 8 early
    1MB DMAs, so the main loop never waits on HBM.
  - The bias add is replaced by P = exp(scores) * exp(bias): the host
    precomputes exp(bias^T) slabs; on-chip application is a DVE f16
    tensor_mul at 2x rate (vs f32 PSUM adds at 1x). Score PSUM is
    double-buffered [128,1024] so ACT exp of one half-group overlaps the
    next half-group's score matmuls.
  - exp emitted with bias=-ln(16) for f16 range; cancels in the softmax
    quotient on the host.
  - sigmoid(x) = 0.5*(1+tanh(x/2)): tanh shares exp's ACT table set (one
    table load); 0.5 folded into Wo, +1 into a tensor_scalar.
  - PV is 2x column-tiled: even k-blocks accumulate into o0 (PSUM
    partitions 0:33), odd k-blocks into o1 (partitions 64:97), each with a
    ones column producing softmax denominators in rows 32/96.
  - Stage 3 (emitted inside the NEXT chunk to keep the PE queue fed) gates
    both o halves into og rows 0:33 / 64:97 with rows 33:64 zeroed, then a
    single K=97 matmul sums both halves through the augmented Wo whose
    column 256 passes the denominators -> one contiguous [128, 16*257]
    f16 output DMA per q-chunk, reshaped and divided on the host.
"""

import numpy as np

_STATE = {}

B, N, CQ, H, CH = 1, 2048, 256, 8, 32
NKB = N // 128  # 16 k-blocks of 128 keys
NQC = 4  # q-chunks of 512 queries
QC = N // NQC  # 512
HG = 8  # half-groups of 2 k-blocks per q-chunk
HW2 = N // 2  # 1024 score columns per half-group
LN16 = float(np.log(16.0))


def _build_nc():
    import concourse.bacc as bacc
    import concourse.tile as tile
    from concourse import mybir

    F32 = mybir.dt.float32
    F16 = mybir.dt.float16
    AF = mybir.ActivationFunctionType

    nc = bacc.Bacc("TRN2", target_bir_lowering=False, debug=False, num_devices=H)

    xq_d = nc.dram_tensor("xq", [128, 2 * N], F16, kind="ExternalInput")
    xkv_d = nc.dram_tensor("xkv", [128, 2 * N], F16, kind="ExternalInput")
    wq_d = nc.dram_tensor("wq", [128, 256], F16, kind="ExternalInput")
    wk_d = nc.dram_tensor("wk", [128, 256], F16, kind="ExternalInput")
    wg_d = nc.dram_tensor("wg", [128, 256], F16, kind="ExternalInput")
    wv_d = nc.dram_tensor("wv", [128, 64], F16, kind="ExternalInput")
    wo_d = nc.dram_tensor("wo", [128, 257], F16, kind="ExternalInput")
    # 32 exp(bias) slabs of [128, 1024]: slab s=8c+hg covers q-chunk c,
    # k-blocks 2hg..2hg+1
    eb_d = nc.dram_tensor("eb", [128, 32 * HW2], F16, kind="ExternalInput")
    out_d = nc.dram_tensor("out", [128, 16 * 257], F16, kind="ExternalOutput")

    with tile.TileContext(nc) as tc:
        with (
            tc.tile_pool(name="const", bufs=1) as cpool,
            tc.tile_pool(name="proj", bufs=1) as ppool,
            tc.tile_pool(name="pexp", bufs=2) as pxpool,
            tc.tile_pool(name="pmul", bufs=3) as pmpool,
            tc.tile_pool(name="ogp", bufs=2) as ogpool,
            tc.tile_pool(name="outs", bufs=1) as opool,
        ):
            wq = cpool.tile([128, 256], F16)
            nc.sync.dma_start(out=wq, in_=wq_d[:, :])
            wk = cpool.tile([128, 256], F16)
            nc.sync.dma_start(out=wk, in_=wk_d[:, :])
            xq = cpool.tile([128, 2 * N], F16)
            nc.sync.dma_start(out=xq, in_=xq_d[:, :])
            xkv = cpool.tile([128, 2 * N], F16)
            nc.sync.dma_start(out=xkv, in_=xkv_d[:, :])
            wg = cpool.tile([128, 256], F16)
            nc.sync.dma_start(out=wg, in_=wg_d[:, :])
            wv = cpool.tile([128, 64], F16)
            nc.sync.dma_start(out=wv, in_=wv_d[:, :])
            wo = cpool.tile([128, 257], F16)
            nc.sync.dma_start(out=wo, in_=wo_d[:, :])
            # bias preload: one big SBUF buffer, 8 chunk DMAs (1MB each)
            ebsb = cpool.tile([128, 32 * HW2], F16)
            for cc in range(8):
                nc.sync.dma_start(
                    out=ebsb[:, 4 * HW2 * cc : 4 * HW2 * (cc + 1)],
                    in_=eb_d[:, 4 * HW2 * cc : 4 * HW2 * (cc + 1)],
                )

            nln16 = cpool.tile([128, 1], F32)
            nc.vector.memset(nln16, -LN16)
            actwarm = cpool.tile([128, 1], F32)
            nc.scalar.activation(actwarm, nln16, func=AF.Exp)

            qT4 = ppool.tile([128, N], F16, tag="qT4")
            kT4 = ppool.tile([128, N], F16, tag="kT4")
            gt4 = ppool.tile([128, N], F16, tag="gt4")
            tp1 = ppool.tile([128, N], F16, tag="tp1")
            vhat = ppool.tile([128, NKB * 33], F16, tag="vhat")
            outsb = opool.tile([128, 16 * 257], F16)

            nc.vector.memset(vhat, 1.0)
            nc.vector.memset(tp1[32:33, :], 1.0)
            nc.vector.memset(tp1[96:97, :], 1.0)

            # ---- stage 1: projections (full 128x128 PE mode) ----
            with (
                tc.tile_pool(name="proj_ps", bufs=2, space="PSUM") as proj_ps,
                tc.tile_pool(name="v_ps", bufs=2, space="PSUM") as v_ps,
                nc.named_scope("stage1_proj"),
            ):
                for w, src, dst in ((wq, xq, qT4), (wk, xkv, kT4)):
                    for f in range(4):
                        pp = proj_ps.tile([128, QC], F32, tag="pp")
                        nc.tensor.matmul(
                            pp, w[:, 0:128], src[:, QC * f : QC * (f + 1)],
                            start=True, stop=False,
                        )
                        nc.tensor.matmul(
                            pp, w[:, 128:256], src[:, N + QC * f : N + QC * (f + 1)],
                            start=False, stop=True,
                        )
                        nc.vector.tensor_copy(dst[:, QC * f : QC * (f + 1)], pp)
                # v projection, natural layout [seq, ch] + ones column
                for r in range(NKB):
                    vt = v_ps.tile([128, 32], F32, tag="v")
                    nc.tensor.matmul(
                        vt, xkv[:, 128 * r : 128 * (r + 1)], wv[:, 0:32],
                        start=True, stop=False,
                    )
                    nc.tensor.matmul(
                        vt, xkv[:, N + 128 * r : N + 128 * (r + 1)], wv[:, 32:64],
                        start=False, stop=True,
                    )
                    nc.vector.tensor_copy(vhat[:, 33 * r : 33 * r + 32], vt)
                # g projection -> tanh(0.5 x) (same ACT table set as exp)
                for f in range(4):
                    pp = proj_ps.tile([128, QC], F32, tag="pp")
                    nc.tensor.matmul(
                        pp, wg[:, 0:128], xq[:, QC * f : QC * (f + 1)],
                        start=True, stop=False,
                    )
                    nc.tensor.matmul(
                        pp, wg[:, 128:256], xq[:, N + QC * f : N + QC * (f + 1)],
                        start=False, stop=True,
                    )
                    nc.scalar.activation(
                        gt4[:, QC * f : QC * (f + 1)], pp, func=AF.Tanh, scale=0.5
                    )
                nc.vector.tensor_scalar_add(tp1[0:32, :], gt4[0:32, :], 1.0)
                nc.vector.tensor_scalar_add(tp1[64:96, :], gt4[64:96, :], 1.0)

            # ---- stage 2+3: attention main loop ----
            with (
                tc.tile_pool(name="sc_ps", bufs=2, space="PSUM") as sc_pool,
                tc.tile_pool(name="o_ps", bufs=1, space="PSUM") as o_pool,
                tc.tile_pool(name="s3_ps", bufs=2, space="PSUM") as s3_pool,
                nc.named_scope("stage2_attn"),
            ):
                o_tiles = {}

                def stage3(c):
                    # gating + output projection for finished q-chunk c;
                    # emitted inside the next chunk to keep the PE queue fed.
                    # og rows 0:33 = gated o0, rows 64:97 = gated o1, rows
                    # 33:64 zero -> one K=97 matmul sums both halves.
                    o0, o1 = o_tiles.pop(c)
                    og = ogpool.tile([128, QC], F16, tag="og")
                    nc.vector.memset(og[32:64, :], 0.0)
                    nc.vector.tensor_mul(
                        og[0:33, :], o0, tp1[0:33, QC * c : QC * (c + 1)]
                    )
                    nc.vector.tensor_mul(
                        og[64:97, :], o1, tp1[64:97, QC * c : QC * (c + 1)]
                    )
                    for j in range(4):
                        qb = 4 * c + j
                        s3 = s3_pool.tile([128, 257], F32, tag="s3")
                        nc.tensor.matmul(
                            s3, og[0:97, 128 * j : 128 * (j + 1)], wo[0:97, :],
                            start=True, stop=True,
                        )
                        nc.vector.tensor_copy(
                            outsb[:, 257 * qb : 257 * (qb + 1)], s3
                        )
                    nc.sync.dma_start(
                        out=out_d[:, 257 * 4 * c : 257 * 4 * (c + 1)],
                        in_=outsb[:, 257 * 4 * c : 257 * 4 * (c + 1)],
                    )

                for c in range(NQC):
                    o0 = o_pool.tile([33, QC], F32, tag="o0")
                    o1_full = o_pool.tile([128, QC], F32, tag="o1")
                    o1 = o1_full[64:97, :]
                    o_tiles[c] = (o0, o1)
                    for hg in range(HG):
                        s = HG * c + hg
                        ebs = ebsb[:, HW2 * s : HW2 * (s + 1)]
                        sc = sc_pool.tile([128, HW2], F32, tag="sc")
                        for i in range(2):
                            kb = 2 * hg + i
                            nc.tensor.matmul(
                                sc[:, 512 * i : 512 * (i + 1)],
                                kT4[32 * i : 32 * (i + 1), 128 * kb : 128 * (kb + 1)],
                                qT4[32 * i : 32 * (i + 1), QC * c : QC * (c + 1)],
                                start=True, stop=True,
                                tile_position=(32 * i, 0),
                            )
                        if hg == 0 and c > 0:
                            stage3(c - 1)
                        pexp = pxpool.tile([128, HW2], F16, tag="pexp")
                        nc.scalar.activation(pexp, sc, func=AF.Exp, bias=nln16)
                        pt = pmpool.tile([128, HW2], F16, tag="p")
                        nc.vector.tensor_mul(pt, pexp, ebs)
                        for i in range(2):
                            kb = 2 * hg + i
                            nc.tensor.matmul(
                                o0 if i == 0 else o1,
                                vhat[:, 33 * kb : 33 * kb + 33],
                                pt[:, 512 * i : 512 * (i + 1)],
                                start=(hg == 0),
                                stop=(hg == HG - 1),
                                tile_position=(0, 0) if i == 0 else (0, 64),
                            )
                stage3(NQC - 1)

    nc.compile()
    return nc


def _get_nc():
    if "nc" not in _STATE:
        _STATE["nc"] = _build_nc()
    return _STATE["nc"]


def _pack2(m, dtype):
    """[256, X] -> [128, 2X]: c-chunk 0 in cols [0:X], chunk 1 in [X:2X]."""
    return np.ascontiguousarray(
        np.concatenate([m[0:128], m[128:256]], axis=1).astype(dtype)
    )


def kernel(q_x, kv_x, attn_bias, Wq, Wk, Wv, Wg, Wo):
    from concourse.bass_utils import run_bass_kernel_spmd

    BF = np.float16
    nc = _get_nc()

    q_x = np.asarray(q_x, dtype=np.float32)
    kv_x = np.asarray(kv_x, dtype=np.float32)
    attn_bias = np.asarray(attn_bias, dtype=np.float32)
    Wq = np.asarray(Wq, dtype=np.float32)
    Wk = np.asarray(Wk, dtype=np.float32)
    Wv = np.asarray(Wv, dtype=np.float32)
    Wg = np.asarray(Wg, dtype=np.float32)
    Wo = np.asarray(Wo, dtype=np.float32)

    xq = _pack2(np.ascontiguousarray(q_x[0].T), BF)
    xkv = _pack2(np.ascontiguousarray(kv_x[0].T), BF)
    scale = np.float32(1.0 / np.sqrt(CH))

    in_maps = []
    for h in range(H):
        sl = slice(CH * h, CH * (h + 1))
        # 32 slabs [128, 1024], slab s=8c+hg covers q-chunk c, k-blocks
        # 2hg..2hg+1: slab[p, 512i+j] = bT[128*(2hg+i)+p, 512c+j].
        # Even hg slabs carry raw bias (PE eye-add), odd hg exp(bias) (DVE).
        bT = attn_bias[0, h].T.astype(np.float32)  # [keys, queries]
        slabs = (
            bT.reshape(8, 2, 128, 4, 512)  # hg, i, p, c, j
            .transpose(3, 0, 2, 1, 4)  # c, hg, p, i, j
            .reshape(32, 128, HW2)
        ).copy()
        slabs = np.exp(slabs)
        eb = np.ascontiguousarray(
            slabs.astype(BF).transpose(1, 0, 2).reshape(128, 32 * HW2)
        )
        woaug = np.zeros((128, 257), dtype=BF)
        woaug[0:32, 0:256] = (0.5 * Wo[sl, :]).astype(BF)
        woaug[32, 256] = 1.0
        woaug[64:96, 0:256] = woaug[0:32, 0:256]
        woaug[96, 256] = 1.0
        in_maps.append(
            {
                "xq": xq,
                "xkv": xkv,
                "wq": _pack2(np.tile(Wq[:, sl] * scale, (1, 4)), BF),
                "wk": _pack2(np.tile(Wk[:, sl], (1, 4)), BF),
                "wg": _pack2(np.tile(Wg[:, sl], (1, 4)), BF),
                "wv": _pack2(Wv[:, sl], BF),
                "wo": woaug,
                "eb": eb,
            }
        )

    res = run_bass_kernel_spmd(nc, in_maps, list(range(H)))

    out = np.zeros((N, CQ), dtype=np.float32)
    for h in range(H):
        full = (
            res.results[h]["out"]
            .astype(np.float32)
            .reshape(128, 16, 257)
            .transpose(1, 0, 2)
            .reshape(N, 257)
        )
        out += full[:, 0:256] / full[:, 256][:, None]
    return out.reshape(B, N, CQ).astype(np.float32)
